# revision 15
# baseline (speedup 1.0000x reference)
"""Trainium2 Bass kernel for a GroupNorm + cross-attention block.

Reference computation (shapes hardcoded):
  x:[2,512,16,16,16] -> GroupNorm(32 groups over (16ch x 4096 spatial))
  q = xn_seq @ Wq ; k,v = context @ Wk/Wv  (context:[2,1024,768])
  attn = softmax(q k^T / 8) ; out = (attn v) @ Wo + bo + residual
  output: [2,512,16,16,16]

Sharding: 8 cores = 2 batches x 4 sequence-quarters (1024 of 4096 voxel
tokens each). Each core computes all 8 heads for its token slice; the only
cross-core communication is a [8,8] f32 AllReduce of GroupNorm statistics
within each 4-core batch group.

Device layout notes:
 - Everything keeps channels/inner-dim on the partition axis, so no
   transposes are needed anywhere on device: the host feeds context^T.
 - GroupNorm is folded into the q projection: Wq rows are scaled by the
   per-channel A = gamma*rstd and a rank-1 offset q0 = Wq^T B is added,
   so normalized x is never materialized.
 - V and K^T projections are emitted BEFORE the GroupNorm stats ->
   AllReduce -> affine -> q0 chain so the in-order PE queue hides the
   collective round trip behind ~20us of projection matmuls.
 - Softmax runs without max-subtraction but with a constant -2 logit
   bias (cancelled exactly by the shared denominator) so exp() fits
   fp8e4 range; exp is the only Activation-engine work (it is the
   critical engine), all PSUM->SBUF copies go to DVE.
 - The softmax denominator comes from an extra all-ones column appended
   to the V tile (lhsT [128,2,65]) so row 64 of the AV matmul
   accumulates sum(exp) for free.
 - exp outputs and V are bf16 (fp8e4 + DoubleRow was tried and lands
   just over the 2e-2 gate: peaked-attention tokens keep the full ~6%
   fp8 relative error).
 - AV normalization: DVE reciprocal of the PSUM denominator row (f16),
   f16 DRAM bounce to partition-broadcast it (stride-0 partition reads
   are DRAM-only; DVE cannot read two PSUM operands, DMA cannot read
   PSUM, gpsimd partition_broadcast/mul are value-correct but
   Q7-software slow on HW, and custom-DVE reciprocal_approx_fast
   returns garbage), then one DVE multiply against the PSUM numerator.
 - Projection matmuls use float32r (full PE speed at >=256 free dim).
"""

import os
from contextlib import ExitStack, nullcontext

import numpy as np

import concourse.bass as bass
import concourse.mybir as mybir
import concourse.tile as tile
from concourse import bacc, bass_utils

F32 = mybir.dt.float32
F32R = mybir.dt.float32r
F16 = mybir.dt.float16
BF16 = mybir.dt.bfloat16
FP8 = mybir.dt.float8e4
AF = mybir.ActivationFunctionType
ALU = mybir.AluOpType
DR = mybir.MatmulPerfMode.DoubleRow

B = 2
C = 512
N = 4096            # voxel tokens per batch (16*16*16)
NL = 1024           # tokens per core (N / 4)
CTX = 1024
CTXD = 768
HEADS = 8
HD = 64
INNER = HEADS * HD  # 512
GROUPS = 32
EPS = 1e-5
SCALE = HD ** -0.5
EBIAS = -3.5        # constant logit bias, cancels in softmax; keeps
                    # exp() below fp8e4 max (448) out to ~9.6 sigma logits

CT = C // 128       # 4 channel tiles
KT = CTXD // 128    # 6 context-dim tiles
MT = CTX // 128     # 8 ctx row tiles
MP = MT // 2        # 4 ctx row-tile PAIRS (DoubleRow AV)
PAIRS = HEADS // 2  # 4 head-pair tiles (128 rows each)

_CACHED_NC = None


def build_nc(loop_iters=1, skip_collective=False):
    # Bacc (not raw Bass): its finalize() runs the wait-splitting passes
    # (move_matmul_waits_to_ldweights / generate_event_semaphores) that the
    # TRN2 ISA requires — walrus rejects multi-wait matmuls otherwise.
    # loop_iters > 1 wraps the body in a device-side For_i so per-iteration
    # device time can be measured without per-dispatch overhead.
    nc = bacc.Bacc("TRN2", target_bir_lowering=False, debug=False, num_devices=8)

    x_d = nc.dram_tensor("x_l", [CT, 128, NL], F32R, kind="ExternalInput")
    ctxT_d = nc.dram_tensor("ctxT", [KT, 128, CTX], F32R, kind="ExternalInput")
    wq_d = nc.dram_tensor("wq", [CT, 128, INNER], F32R, kind="ExternalInput")
    wk_d = nc.dram_tensor("wk", [KT, 128, INNER], F32R, kind="ExternalInput")
    wv_d = nc.dram_tensor("wv", [KT, 128, INNER], F32R, kind="ExternalInput")
    wo_d = nc.dram_tensor("wo", [CT, 128, C], F32R, kind="ExternalInput")
    gamma_d = nc.dram_tensor("gamma_t", [128, CT], F32, kind="ExternalInput")
    beta_d = nc.dram_tensor("beta_t", [128, CT], F32, kind="ExternalInput")
    bo_d = nc.dram_tensor("bo_t", [128, CT], F32, kind="ExternalInput")
    self_fwd_d = nc.dram_tensor("sel_fwd", [128, 8], F32, kind="ExternalInput")
    sel_bwd_d = nc.dram_tensor("sel_bwd", [8, 128], F32, kind="ExternalInput")
    out_d = nc.dram_tensor("out_l", [CT, 128, NL], F32, kind="ExternalOutput")

    stats_in_d = nc.dram_tensor("stats_in", [8, 8], F32)
    stats_out_d = nc.dram_tensor("stats_out", [8, 8], F32)
    # scratch for partition-broadcasting softmax reciprocals (DRAM bounce:
    # SBUF/PSUM sources cannot be read with partition-stride 0, DRAM can)
    den_d = nc.dram_tensor("den_scratch", [HEADS, 2, 512], F16)

    with tile.TileContext(nc) as tc, ExitStack() as ctx:
        consts = ctx.enter_context(tc.tile_pool(name="consts", bufs=1))
        wpool = ctx.enter_context(tc.tile_pool(name="weights", bufs=1))
        xpool = ctx.enter_context(tc.tile_pool(name="x", bufs=2))
        cxpool = ctx.enter_context(tc.tile_pool(name="ctx", bufs=1))
        ktpool = ctx.enter_context(tc.tile_pool(name="kt", bufs=1))
        vpool = ctx.enter_context(tc.tile_pool(name="v", bufs=1))
        qpool = ctx.enter_context(tc.tile_pool(name="qt", bufs=1))
        epool = ctx.enter_context(tc.tile_pool(name="e", bufs=8))
        otpool = ctx.enter_context(tc.tile_pool(name="ot", bufs=1))
        spool = ctx.enter_context(tc.tile_pool(name="small", bufs=4))
        dbpool = ctx.enter_context(tc.tile_pool(name="denb", bufs=2))
        opool = ctx.enter_context(tc.tile_pool(name="outs", bufs=3))

        pp = ctx.enter_context(tc.tile_pool(name="pproj", bufs=2, space="PSUM"))
        pst = ctx.enter_context(tc.tile_pool(name="pst", bufs=2, space="PSUM"))
        pav = ctx.enter_context(tc.tile_pool(name="pav", bufs=2, space="PSUM"))

        def _body():

            # ---- DMA loads, ordered for steady-state For_i pipelining ----
            # Free-flowing reloads (consumers finish early in the previous
            # iteration) go first; tiles whose last use is at the iteration
            # tail (x: residual; wo: output projection; bo: bias add) go
            # last so their WAR waits don't head-block the SP queue.
            sel_fwd = consts.tile([128, 8], F32, tag="sel_fwd")
            nc.sync.dma_start(out=sel_fwd, in_=self_fwd_d[:, :])
            sel_bwd = consts.tile([8, 128], F32, tag="sel_bwd")
            nc.sync.dma_start(out=sel_bwd, in_=sel_bwd_d[:, :])
            gamma_sb = consts.tile([128, CT], F32, tag="gamma")
            nc.sync.dma_start(out=gamma_sb, in_=gamma_d[:, :])
            beta_sb = consts.tile([128, CT], F32, tag="beta")
            nc.sync.dma_start(out=beta_sb, in_=beta_d[:, :])
            eps_sb = consts.tile([128, 1], F32, tag="eps")
            nc.vector.memset(eps_sb, EPS)
            ebias_sb = consts.tile([128, 1], F32, tag="ebias")
            nc.vector.memset(ebias_sb, EBIAS)

            ctx_sb = []
            for kk in range(KT):
                ct_ = cxpool.tile([128, CTX], F32R, tag=f"ctx{kk}")
                nc.sync.dma_start(out=ct_, in_=ctxT_d[kk])
                ctx_sb.append(ct_)
            wv_sb, wk_sb, wq_sb, wo_sb = [], [], [], []
            for kk in range(KT):
                w = wpool.tile([128, INNER], F32R, tag=f"wv{kk}")
                nc.sync.dma_start(out=w, in_=wv_d[kk])
                wv_sb.append(w)
            for kk in range(KT):
                w = wpool.tile([128, INNER], F32R, tag=f"wk{kk}")
                nc.sync.dma_start(out=w, in_=wk_d[kk])
                wk_sb.append(w)
            for t in range(CT):
                w = wpool.tile([128, INNER], F32R, tag=f"wq{t}")
                nc.sync.dma_start(out=w, in_=wq_d[t])
                wq_sb.append(w)
            x_sb = []
            for t in range(CT):
                xt = xpool.tile([128, NL], F32R, tag=f"x{t}")
                nc.sync.dma_start(out=xt, in_=x_d[t])
                x_sb.append(xt)
            for t in range(CT):
                w = wpool.tile([128, C], F32R, tag=f"wo{t}")
                nc.sync.dma_start(out=w, in_=wo_d[t])
                wo_sb.append(w)
            bo_sb = consts.tile([128, CT], F32, tag="bo")
            nc.sync.dma_start(out=bo_sb, in_=bo_d[:, :])

            # ---- V = context @ Wv, packed for DoubleRow AV ----
            # v layout [128ctx, 2(m-parity), 8 heads, 64+1]; col 64 is the
            # all-ones softmax-denominator column.
            v_sb = [
                vpool.tile(
                    [128, 2, HEADS, HD + 1], BF16, tag=f"v{mp}", name=f"v{mp}"
                )
                for mp in range(MP)
            ]
            for mp in range(MP):
                nc.vector.memset(v_sb[mp][:, :, :, HD:HD + 1], 1.0)
            for m in range(MT):
                pv = pp.tile([128, 512], F32, tag="proj")
                for kk in range(KT):
                    nc.tensor.matmul(
                        pv,
                        lhsT=(ctx_sb[kk][:, m * 128:(m + 1) * 128]),
                        rhs=(wv_sb[kk]),
                        start=(kk == 0), stop=(kk == KT - 1),
                    )
                nc.vector.tensor_copy(
                    out=v_sb[m // 2][:, m % 2, :, 0:HD],
                    in_=pv.rearrange("p (h d) -> p h d", h=HEADS),
                )

            # ---- GroupNorm statistics ----
            # per-channel (mean, E[x^2]) over the local token slice, group-
            # reduced on the PE with sel_fwd (value 1/64: 16ch x 4 cores),
            # then AllReduced within the batch group. Emitted after the V
            # section so the V/K projections hide the round trip.
            ps_stats = pav.tile([128, 16], F32, tag="avp")
            for t in range(CT):
                st6 = spool.tile([128, 2, 6], F32, tag="bn6")
                for sg in range(2):
                    nc.vector.bn_stats(
                        out=st6[:, sg, :], in_=x_sb[t][:, sg * 512:(sg + 1) * 512]
                    )
                mv = spool.tile([128, 2], F32, tag="mv")
                nc.vector.bn_aggr(out=mv, in_=st6)
                s12 = spool.tile([128, 2], F32, tag="s12")
                nc.vector.tensor_copy(out=s12[:, 0:1], in_=mv[:, 0:1])
                nc.vector.tensor_mul(s12[:, 1:2], mv[:, 0:1], mv[:, 0:1])
                nc.vector.tensor_add(s12[:, 1:2], s12[:, 1:2], mv[:, 1:2])
                nc.tensor.matmul(
                    ps_stats[0:8, t * 2:t * 2 + 2], lhsT=sel_fwd, rhs=s12,
                    start=True, stop=True,
                )
            stats_sb = spool.tile([8, 8], F32, tag="gst")
            nc.vector.tensor_copy(out=stats_sb, in_=ps_stats[0:8, 0:8])
            nc.sync.dma_start(out=stats_in_d[:, :], in_=stats_sb)
            if skip_collective:
                # timing-only variant: collectives inside a device-side For_i
                # desync the mesh on the 2nd iteration, so the timing loop
                # substitutes a local DRAM copy (output values are wrong by a
                # constant stats factor; latency profile is comparable).
                nc.sync.dma_start(out=stats_out_d[:, :], in_=stats_in_d[:, :])
            else:
                nc.gpsimd.collective_compute(
                    "AllReduce",
                    ALU.add,
                    replica_groups=[[0, 1, 2, 3], [4, 5, 6, 7]],
                    ins=[stats_in_d[:, :]],
                    outs=[stats_out_d[:, :]],
                )
            g_sb = spool.tile([8, 8], F32, tag="gout")
            nc.sync.dma_start(out=g_sb, in_=stats_out_d[:, :])

            # ---- K^T = (context @ Wk)^T  (per head-pair tile) ----
            kT_sb = []
            for j in range(PAIRS):
                kt_ = ktpool.tile([128, CTX], BF16, tag=f"kT{j}")
                for cc in range(2):
                    pk = pp.tile([128, 512], F32, tag="proj")
                    for kk in range(KT):
                        nc.tensor.matmul(
                            pk,
                            lhsT=(wk_sb[kk][:, j * 128:(j + 1) * 128]),
                            rhs=(ctx_sb[kk][:, cc * 512:(cc + 1) * 512]),
                            start=(kk == 0), stop=(kk == KT - 1),
                        )
                    nc.vector.tensor_copy(
                        out=kt_[:, cc * 512:(cc + 1) * 512], in_=pk
                    )
                kT_sb.append(kt_)

            # Per channel-tile affine coefficients:
            #   A = gamma * rstd,  B = beta - mean*rstd*gamma
            # rstd = exp(-0.5 * ln(var + eps)) keeps everything in the single
            # natural_log_exp activation table set shared with the softmax exp.
            cA, cB = [], []
            for t in range(CT):
                m1 = g_sb[:, 2 * t:2 * t + 1]
                m2 = g_sb[:, 2 * t + 1:2 * t + 2]
                var8 = spool.tile([8, 1], F32, tag=f"var{t}")
                nc.vector.tensor_mul(var8, m1, m1)
                nc.vector.tensor_sub(var8, m2, var8)
                nc.scalar.activation(var8, var8, AF.Ln, bias=eps_sb[0:8, :])
                nc.scalar.activation(var8, var8, AF.Exp, scale=-0.5)  # rstd
                ab8 = spool.tile([8, 2], F32, tag=f"ab{t}")
                nc.vector.tensor_copy(out=ab8[:, 0:1], in_=var8)
                nc.vector.tensor_mul(ab8[:, 1:2], m1, var8)  # mean*rstd
                ps_ab = pav.tile([128, 16], F32, tag="avp")
                nc.tensor.matmul(
                    ps_ab[:, 0:2], lhsT=sel_bwd, rhs=ab8, start=True, stop=True
                )
                a_t = spool.tile([128, 1], F32, tag=f"cA{t}")
                b_t = spool.tile([128, 1], F32, tag=f"cB{t}")
                nc.vector.tensor_mul(a_t, ps_ab[:, 0:1], gamma_sb[:, t:t + 1])
                nc.vector.tensor_mul(b_t, ps_ab[:, 1:2], gamma_sb[:, t:t + 1])
                nc.vector.tensor_sub(b_t, beta_sb[:, t:t + 1], b_t)
                cA.append(a_t)
                cB.append(b_t)

            # q0 = Wq^T B per head-pair tile (before Wq is scaled in place).
            q0_sb = []
            for j in range(PAIRS):
                pq0 = pav.tile([128, 16], F32, tag="avp")
                for t in range(CT):
                    nc.tensor.matmul(
                        pq0[:, 0:1],
                        lhsT=wq_sb[t][:, j * 128:(j + 1) * 128].bitcast(F32),
                        rhs=cB[t],
                        start=(t == 0), stop=(t == CT - 1),
                    )
                q0 = spool.tile([128, 1], F32, tag=f"q0{j}")
                nc.vector.tensor_copy(out=q0, in_=pq0[:, 0:1])
                q0_sb.append(q0)
            # Fold A into Wq rows in place (q0 above already consumed raw Wq).
            for t in range(CT):
                nc.vector.tensor_scalar_mul(
                    out=wq_sb[t], in0=wq_sb[t], scalar1=cA[t]
                )

            # ---- q^T (per head-pair tile), GroupNorm pre-folded ----
            qT_sb = []
            for j in range(PAIRS):
                qt_ = qpool.tile([128, NL], BF16, tag=f"qT{j}")
                for ncc in range(2):
                    pq = pp.tile([128, 512], F32, tag="proj")
                    for t in range(CT):
                        nc.tensor.matmul(
                            pq,
                            lhsT=(wq_sb[t][:, j * 128:(j + 1) * 128]),
                            rhs=(x_sb[t][:, ncc * 512:(ncc + 1) * 512]),
                            start=(t == 0), stop=(t == CT - 1),
                        )
                    nc.vector.tensor_scalar_add(
                        out=qt_[:, ncc * 512:(ncc + 1) * 512],
                        in0=pq,
                        scalar1=q0_sb[j],
                    )
                qT_sb.append(qt_)

            # ---- attention per (head-pair, token-chunk of 512) ----
            # Scores for both heads of a pair are issued back-to-back with
            # tile_position (0,0)/(64,0): K=64 matmuls on distinct PE array
            # row-groups run concurrently (~2x). One exp covers both heads
            # and writes fp8 into the DoubleRow-packed e tile.
            ot_sb = [
                otpool.tile([128, NL], F32R, tag=f"ot{j}", name=f"ot{j}")
                for j in range(PAIRS)
            ]
            for ncc in range(2):
                for j in range(PAIRS):
                    cs = slice(ncc * 512, (ncc + 1) * 512)
                    e_tiles = []
                    for mp in range(MP):
                        et = epool.tile([128, 2, 1024], BF16, tag="e")
                        for par in range(2):
                            m = 2 * mp + par
                            stp = pst.tile([128, 1024], F32, tag="st")
                            nc.tensor.matmul(
                                stp[:, 0:512],
                                lhsT=kT_sb[j][0:HD, m * 128:(m + 1) * 128],
                                rhs=qT_sb[j][0:HD, cs],
                                start=True, stop=True,
                            )
                            nc.tensor.matmul(
                                stp[:, 512:1024],
                                lhsT=kT_sb[j][HD:128, m * 128:(m + 1) * 128],
                                rhs=qT_sb[j][HD:128, cs],
                                start=True, stop=True,
                            )
                            nc.scalar.activation(
                                et[:, par, :], stp, AF.Exp, scale=SCALE,
                                bias=ebias_sb,
                            )
                        e_tiles.append(et)
                    for half in range(2):
                        h = 2 * j + half
                        rs = slice(half * HD, (half + 1) * HD)
                        es = slice(half * 512, (half + 1) * 512)
                        avp = pav.tile([128, 512], F32, tag="avp")
                        for m in range(MT):
                            nc.tensor.matmul(
                                avp[0:HD + 1, :],
                                lhsT=v_sb[m // 2][:, m % 2, h, :],
                                rhs=e_tiles[m // 2][:, m % 2, es],
                                start=(m == 0), stop=(m == MT - 1),
                            )
                        rec = dbpool.tile([1, 512], F16, tag="rec")
                        with nc.allow_low_precision(
                            reason="f16 softmax-denominator reciprocal; "
                            "5e-4 rel is far inside the error budget"
                        ):
                            nc.vector.reciprocal(
                                out=rec, in_=avp[HD:HD + 1, :]
                            )
                        nc.sync.dma_start(out=den_d[h, ncc, :], in_=rec)
                        den_row = den_d[h, ncc, :]
                        den_bc_ap = bass.AP(
                            tensor=den_row.tensor,
                            offset=den_row.offset,
                            ap=[[0, HD], [1, 512]],
                        )
                        den_sb = dbpool.tile([HD, 512], F16, tag="den")
                        nc.sync.dma_start(out=den_sb, in_=den_bc_ap)
                        nc.vector.tensor_mul(
                            ot_sb[j][rs, cs], avp[0:HD, :], den_sb
                        )

                cs = slice(ncc * 512, (ncc + 1) * 512)
                # out = OT^T Wo + bo + residual for this token chunk; overlaps
                # with the next chunk's attention on other engines.
                for t in range(CT):
                    po = pp.tile([128, 512], F32, tag="proj")
                    for jj in range(PAIRS):
                        nc.tensor.matmul(
                            po,
                            lhsT=(wo_sb[jj][:, t * 128:(t + 1) * 128]),
                            rhs=(ot_sb[jj][:, cs]),
                            start=(jj == 0), stop=(jj == PAIRS - 1),
                        )
                    res = opool.tile([128, 512], F32, tag="res")
                    nc.vector.scalar_tensor_tensor(
                        out=res,
                        in0=po,
                        scalar=bo_sb[:, t:t + 1],
                        in1=x_sb[t][:, cs],
                        op0=ALU.add,
                        op1=ALU.add,
                    )
                    nc.sync.dma_start(out=out_d[t, :, cs], in_=res)

        # Timing variant: the For_i semaphore-reset block carries an
        # InstAllEngineBarrier, so consecutive iterations cannot overlap.
        # Unrolling two bodies per For_i iteration halves the barrier count
        # and lets pool-buffer rotation (x: bufs=2) pipeline the unrolled
        # pair for a true steady-state measurement.
        if loop_iters <= 1:
            _body()
        else:
            for _ in range(loop_iters % 2):
                _body()
            with tc.For_i(0, loop_iters // 2, 1):
                _body()
                _body()

    nc.finalize()
    return nc


def _host_prep(x, context, gamma, beta, Wq, Wk, Wv, Wo, bo):
    """Build the 8 per-core input maps (host-side slicing/transposes only)."""
    x2 = np.ascontiguousarray(x, np.float32).reshape(B, C, N)
    ctx = np.ascontiguousarray(context, np.float32)

    sel_fwd = np.zeros((128, 8), np.float32)
    for p in range(128):
        sel_fwd[p, p // 16] = 1.0 / 64.0  # 16 channels x 4 cores
    sel_bwd = np.zeros((8, 128), np.float32)
    for p in range(128):
        sel_bwd[p // 16, p] = 1.0

    shared = {
        "wq": np.ascontiguousarray(Wq, np.float32).reshape(CT, 128, INNER),
        "wk": np.ascontiguousarray(Wk, np.float32).reshape(KT, 128, INNER),
        "wv": np.ascontiguousarray(Wv, np.float32).reshape(KT, 128, INNER),
        "wo": np.ascontiguousarray(Wo, np.float32).reshape(CT, 128, C),
        "gamma_t": np.ascontiguousarray(
            np.asarray(gamma, np.float32).reshape(CT, 128).T
        ),
        "beta_t": np.ascontiguousarray(
            np.asarray(beta, np.float32).reshape(CT, 128).T
        ),
        "bo_t": np.ascontiguousarray(np.asarray(bo, np.float32).reshape(CT, 128).T),
        "sel_fwd": sel_fwd,
        "sel_bwd": sel_bwd,
        "stats_in": np.zeros((8, 8), np.float32),
        "stats_out": np.zeros((8, 8), np.float32),
    }

    in_maps = []
    for core in range(8):
        b, qt = core // 4, core % 4
        m = dict(shared)
        m["x_l"] = np.ascontiguousarray(
            x2[b, :, qt * NL:(qt + 1) * NL]
        ).reshape(CT, 128, NL)
        m["ctxT"] = np.ascontiguousarray(ctx[b].T).reshape(KT, 128, CTX)
        in_maps.append(m)
    return in_maps


def _assemble(results):
    out = np.zeros((B, C, N), np.float32)
    for core in range(8):
        b, qt = core // 4, core % 4
        out[b, :, qt * NL:(qt + 1) * NL] = results[core]["out_l"].reshape(C, NL)
    return out.reshape(B, C, 16, 16, 16)


def run(inputs, trace=False):
    global _CACHED_NC
    if _CACHED_NC is None:
        _CACHED_NC = build_nc()
    nc = _CACHED_NC
    in_maps = _host_prep(**inputs)
    # stats_in/stats_out are internal dram tensors, not ExternalInputs
    for m in in_maps:
        m.pop("stats_in")
        m.pop("stats_out")
    bkr = bass_utils.run_bass_kernel_spmd(
        nc, in_maps, list(range(8)), trace=trace
    )
    return _assemble(bkr.results), bkr


def kernel(**inputs):
    out, _ = run(inputs)
    return out
